# revision 53
# baseline (speedup 1.0000x reference)
"""Trainium2 Bass kernel for nn_LlamaAttention_cam (sparse_attention).

Sharding: 16 heads across 8 cores (2 heads/core), both batches per core.
Q/K/V projections column-parallel over heads, o_proj row-parallel (per-core
partial outputs summed on host). The CaM merge is a rank-1 correction
applied on host from tiny device-side statistics.

Key optimizations over the bf16 baseline (387us -> 285us cost-model time):
- QKV and o_proj matmuls run in error-compensated fp8: each operand is
  split host-side (or on the normalize path) into e4m3 hi + lo parts and
  the product takes 3 of the 4 cross terms as DoubleRow matmuls, giving
  0.75x the bf16 PE cost at ~bf16 accuracy (plain fp8 measured 5-10x
  over the error budget; hi+lo lands at ~1.3e-2 vs the 2e-2 gate).
- Softmax row-sums cost ~1 PE cycle per 128x128 P block: P quarters are
  the *stationary* operand with a 1-column ones vector moving (matmul
  cost scales with moving columns only).  Per-query reciprocals are
  PE-transposed into a partition-0 row and applied via one partition
  broadcast.
- Causal trimming at 128-column granularity on scores/exp/PV/rowsums.
- Scores/exp run one wide tile ahead of mask/PV; the sibling head's
  normalize chain and the previous chunk's o_proj are emitted inside the
  next attention chunk so the PE rarely waits on Act/DVE/Pool chains.
- DMAs are issued in deadline order (first hs chunk + hi weights first,
  cold constants last); weights are host-preswizzled so every transfer
  moves contiguous >=512B descriptors at full DMA rate.

Self-contained: hardcodes all shapes; takes full inputs, returns full output.
"""

import math
import os

import numpy as np
import ml_dtypes

B, T, HID, H = 2, 2048, 2048, 16
D = 128
NCORES = 8
HL = H // NCORES  # heads per core = 2
BT = B * T  # 4096
NF = HID // 128  # 16 f-tiles
SCALE = 1.0 / math.sqrt(D)
RB = int(0.25 * T)  # 512 recent budget
WS = T - RB  # 1536
EVICT = WS - 1  # 1535

S_HS = 8.0
S_W = 256.0
DESC = 1.0 / (S_HS * S_W)  # 1/2048
S_OT = 16.0
S_WO = 256.0
DESC_O = 1.0 / (S_OT * S_WO)  # 1/4096

# jax.random.uniform(jax.random.key(42), (2,16), float32); bernoulli(key,p) == u < p
U_CONST = np.array(
    [[0.59400654, 0.43801308, 0.6285691, 0.00791204, 0.27834702,
      0.7976179, 0.8521497, 0.9625306, 0.67656493, 0.11104441,
      0.4959929, 0.7311437, 0.18970704, 0.1544199, 0.03802836,
      0.33559263],
     [0.92825687, 0.6123972, 0.49262476, 0.733806, 0.18920851,
      0.15386605, 0.037136197, 0.32930005, 0.9372028, 0.5957513,
      0.4615929, 0.6695677, 0.07019377, 0.39408123, 0.55786455,
      0.35412872]], dtype=np.float32)

BF16 = ml_dtypes.bfloat16
F8 = ml_dtypes.float8_e4m3

_NC_CACHE = {}


def build_nc():
    import concourse.bacc as bacc
    import concourse.mybir as mybir
    import concourse.tile as tile

    f32 = mybir.dt.float32
    bf16 = mybir.dt.bfloat16
    fp8 = mybir.dt.float8e4
    EXP = mybir.ActivationFunctionType.Exp
    COPY = mybir.ActivationFunctionType.Copy
    DR = mybir.MatmulPerfMode.DoubleRow

    nc = bacc.Bacc("TRN2", target_bir_lowering=False, debug=False)

    hs_hi = nc.dram_tensor("hs_hi", [HID, BT], fp8, kind="ExternalInput")
    hs_lo = nc.dram_tensor("hs_lo", [HID, BT], fp8, kind="ExternalInput")
    w8 = {}
    for nm in ("wq", "wk", "wv"):
        for part in ("hi", "lo"):
            # host pre-swizzled to the SBUF layout: [p, nf*256] contiguous
            w8[f"{nm}_{part}"] = nc.dram_tensor(
                f"{nm}_{part}", [128, NF * 256], fp8, kind="ExternalInput")
    wo_hi = nc.dram_tensor("wo_hi", [128, 2 * HID], fp8, kind="ExternalInput")
    wo_lo = nc.dram_tensor("wo_lo", [128, 2 * HID], fp8, kind="ExternalInput")
    cosd = nc.dram_tensor("cosT", [128, T], bf16, kind="ExternalInput")
    sind = nc.dram_tensor("sinT", [128, T], bf16, kind="ExternalInput")
    maskd = nc.dram_tensor("masks", [128, 2048], bf16, kind="ExternalInput")
    identd = nc.dram_tensor("ident", [128, 128], bf16, kind="ExternalInput")

    outT = nc.dram_tensor("outT", [HID, BT], bf16, kind="ExternalOutput")
    abard = nc.dram_tensor("abar", [4, 128, 16], bf16, kind="ExternalOutput")
    # per (p): cols 0:4 = chunk-3 rowsums, cols 4:8 = chunk-3 tail sums
    dentd = nc.dram_tensor("dent", [4, 128, 8], f32, kind="ExternalOutput")

    with tile.TileContext(nc) as tc:
        with (
            tc.tile_pool(name="singles", bufs=1) as singles,
            tc.tile_pool(name="res", bufs=1) as res,
            tc.tile_pool(name="stats", bufs=1) as stats,
        ):
            # --- allocate constant tiles (DMAs emitted below, ordered) ---
            w8_sb = {key: singles.tile([128, NF, 256], fp8, tag=key, name=key)
                     for key in w8}
            wo_hi_sb = singles.tile([128, 2, HID], fp8, tag="wohi")
            wo_lo_sb = singles.tile([128, 2, HID], fp8, tag="wolo")
            cos_sb = singles.tile([128, T], bf16, tag="cos")
            sin_sb = singles.tile([128, T], bf16, tag="sin")
            mask_sb = singles.tile([128, 4, 512], bf16, tag="mask")
            ident_sb = singles.tile([128, 128], bf16, tag="ident")
            ones1 = singles.tile([128, 1], bf16, tag="ones1")
            nc.vector.memset(ones1, 1.0)

            # --- residents ---
            qt = [res.tile([128, BT], bf16, tag=f"qt{h}", name=f"qt{h}")
                  for h in range(HL)]
            kt = [res.tile([128, BT], bf16, tag=f"kt{h}", name=f"kt{h}")
                  for h in range(HL)]
            vres = res.tile([128, 32, 256], bf16, tag="vres")
            ot_hi = [[res.tile([128, 2, 512], fp8, tag=f"oth{b}_{c}",
                               name=f"oth{b}_{c}")
                      for c in range(4)] for b in range(2)]
            ot_lo = [[res.tile([128, 2, 512], fp8, tag=f"otl{b}_{c}",
                               name=f"otl{b}_{c}")
                      for c in range(4)] for b in range(2)]
            abar_raw = [stats.tile([128, 16], bf16, tag=f"ab{p}",
                                   name=f"ab{p}") for p in range(4)]

            # ============ Phase 1: QKV projections (comp-fp8) + RoPE ========
            with (
                tc.tile_pool(name="hsp", bufs=2) as hsp,
                tc.tile_pool(name="rope", bufs=4) as rope,
                tc.tile_pool(name="qkps", bufs=4, space="PSUM") as qkps,
                tc.tile_pool(name="vps", bufs=4, space="PSUM") as vps,
            ):
                def load_hs_half(c, which):
                    cs = slice(c * 512, (c + 1) * 512)
                    src = hs_hi if which == "hshi" else hs_lo
                    t = hsp.tile([128, NF, 512], fp8, tag=which, name=which)
                    nc.sync.dma_start(
                        out=t,
                        in_=src[:, cs].rearrange("(nf p) t -> p nf t", p=128))
                    return t

                def load_w(key):
                    nc.sync.dma_start(
                        out=w8_sb[key],
                        in_=w8[key].rearrange("p (nf d) -> p nf d", d=256))

                # DMA order tuned so the PE never starves at kernel start.
                # All transfers share the DMA engines, so everything goes on
                # the sync queue in deadline order: hs_hi(c0), hi weights,
                # lo weights, hs_lo(c0), hs(c1), then the cold constants
                # (cos/sin feed only the DVE rope; masks/ident/wo are needed
                # far later).
                def load_trig(v):
                    vs = slice(v * 512, (v + 1) * 512)
                    nc.sync.dma_start(out=cos_sb[:, vs], in_=cosd[:, vs])
                    nc.sync.dma_start(out=sin_sb[:, vs], in_=sind[:, vs])

                nc.sync.dma_start(
                    out=w8_sb["wq_hi"][:, 0:8, :],
                    in_=w8["wq_hi"][:, 0:2048].rearrange(
                        "p (nf d) -> p nf d", d=256))
                cs0 = slice(0, 512)
                hi0 = hsp.tile([128, NF, 512], fp8, tag="hshi", name="hshi")
                nc.sync.dma_start(
                    out=hi0[:, 0:8, :],
                    in_=hs_hi[0:1024, cs0].rearrange(
                        "(nf p) t -> p nf t", p=128))
                nc.sync.dma_start(
                    out=hi0[:, 8:16, :],
                    in_=hs_hi[1024:2048, cs0].rearrange(
                        "(nf p) t -> p nf t", p=128))
                nc.sync.dma_start(
                    out=w8_sb["wq_hi"][:, 8:16, :],
                    in_=w8["wq_hi"][:, 2048:4096].rearrange(
                        "p (nf d) -> p nf d", d=256))
                for key in ("wk_hi", "wv_hi", "wq_lo", "wk_lo", "wv_lo"):
                    load_w(key)
                load_trig(0)
                lo0 = load_hs_half(0, "hslo")
                hs_tiles = (hi0, lo0)
                load_trig(1)
                hs_next = (load_hs_half(1, "hshi"), load_hs_half(1, "hslo"))
                load_trig(2)
                load_trig(3)
                nc.sync.dma_start(out=ident_sb, in_=identd[:, :])
                nc.sync.dma_start(
                    out=mask_sb, in_=maskd.rearrange("p (v t) -> p v t", v=4))
                nc.sync.dma_start(
                    out=wo_hi_sb, in_=wo_hi.rearrange("p (kt f) -> p kt f",
                                                      f=HID))
                nc.sync.dma_start(
                    out=wo_lo_sb, in_=wo_lo.rearrange("p (kt f) -> p kt f",
                                                      f=HID))

                for c in range(8):
                    cs = slice(c * 512, (c + 1) * 512)
                    hs_hi_t, hs_lo_t = hs_tiles
                    hs_tiles = hs_next
                    if c < 6:
                        hs_next = (load_hs_half(c + 2, "hshi"),
                                   load_hs_half(c + 2, "hslo"))
                    # Three passes with open PSUM groups: A1 = w_hi*hs_hi,
                    # A2 = w_lo*hs_hi, B = w_hi*hs_lo.  This relaxes the
                    # arrival deadlines of the lo weights and hs_lo DMAs.
                    qk_ps = {}
                    v_ps = []
                    for h in range(HL):
                        hsl = slice(h * 128, (h + 1) * 128)
                        for nm in ("wq", "wk"):
                            ps = qkps.tile([128, 512], f32, tag="qk")
                            qk_ps[(nm, h)] = ps
                            for s in range(8):
                                nc.tensor.matmul(
                                    ps,
                                    lhsT=w8_sb[f"{nm}_hi"][:, 2 * s:2 * s + 2, hsl],
                                    rhs=hs_hi_t[:, 2 * s:2 * s + 2, :],
                                    start=(s == 0), stop=False, perf_mode=DR)
                    for sblk in range(4):
                        tsl = slice(sblk * 128, (sblk + 1) * 128)
                        vp = vps.tile([128, 512], f32, tag="v")
                        v_ps.append(vp)
                        for s in range(8):
                            nc.tensor.matmul(
                                vp[:, 0:256],
                                lhsT=hs_hi_t[:, 2 * s:2 * s + 2, tsl],
                                rhs=w8_sb["wv_hi"][:, 2 * s:2 * s + 2, :],
                                start=(s == 0), stop=False, perf_mode=DR)
                    # pass A2
                    for h in range(HL):
                        hsl = slice(h * 128, (h + 1) * 128)
                        for nm in ("wq", "wk"):
                            for s in range(8):
                                nc.tensor.matmul(
                                    qk_ps[(nm, h)],
                                    lhsT=w8_sb[f"{nm}_lo"][:, 2 * s:2 * s + 2, hsl],
                                    rhs=hs_hi_t[:, 2 * s:2 * s + 2, :],
                                    start=False, stop=False, perf_mode=DR)
                    for sblk in range(4):
                        tsl = slice(sblk * 128, (sblk + 1) * 128)
                        for s in range(8):
                            nc.tensor.matmul(
                                v_ps[sblk][:, 0:256],
                                lhsT=hs_hi_t[:, 2 * s:2 * s + 2, tsl],
                                rhs=w8_sb["wv_lo"][:, 2 * s:2 * s + 2, :],
                                start=False, stop=False, perf_mode=DR)
                    # pass B + RoPE / V-copy epilogues
                    tl = slice((c % 4) * 512, (c % 4) * 512 + 512)
                    for h in range(HL):
                        hsl = slice(h * 128, (h + 1) * 128)
                        for nm, dest in (("wq", qt[h]), ("wk", kt[h])):
                            ps = qk_ps[(nm, h)]
                            for s in range(8):
                                nc.tensor.matmul(
                                    ps,
                                    lhsT=w8_sb[f"{nm}_hi"][:, 2 * s:2 * s + 2, hsl],
                                    rhs=hs_lo_t[:, 2 * s:2 * s + 2, :],
                                    start=False, stop=(s == 7), perf_mode=DR)
                            qf = rope.tile([128, 512], bf16, tag="qf")
                            if c == 7:
                                nc.vector.tensor_scalar_mul(qf, ps, DESC)
                            else:
                                nc.scalar.activation(qf, ps, COPY, scale=DESC)
                            rot = rope.tile([128, 512], bf16, tag="rot")
                            nc.gpsimd.dma_start(out=rot[0:64, :], in_=qf[64:128, :])
                            nc.gpsimd.dma_start(out=rot[64:128, :], in_=qf[0:64, :])
                            t1 = rope.tile([128, 512], bf16, tag="t1")
                            nc.vector.tensor_mul(t1, rot, sin_sb[:, tl])
                            t2 = rope.tile([128, 512], bf16, tag="t2")
                            nc.vector.tensor_mul(t2, qf, cos_sb[:, tl])
                            nc.vector.tensor_add(dest[:, cs], t1, t2)
                    for sblk in range(4):
                        tsl = slice(sblk * 128, (sblk + 1) * 128)
                        vp = v_ps[sblk]
                        for s in range(8):
                            nc.tensor.matmul(
                                vp[:, 0:256],
                                lhsT=hs_lo_t[:, 2 * s:2 * s + 2, tsl],
                                rhs=w8_sb["wv_hi"][:, 2 * s:2 * s + 2, :],
                                start=False, stop=(s == 7), perf_mode=DR)
                        if c == 7:
                            nc.vector.tensor_scalar_mul(
                                vres[:, c * 4 + sblk, :], vp[:, 0:256], DESC)
                        else:
                            nc.scalar.activation(
                                vres[:, c * 4 + sblk, :], vp[:, 0:256], COPY,
                                scale=DESC)

            # ========== Phase 2+3: attention + interleaved o_proj ==========
            with (
                tc.tile_pool(name="sps", bufs=2, space="PSUM") as sps,
                tc.tile_pool(name="ops", bufs=3, space="PSUM") as ops,
                tc.tile_pool(name="denps", bufs=1, space="PSUM") as denps,
                tc.tile_pool(name="pt", bufs=18) as ptp,
                tc.tile_pool(name="att_sm", bufs=3) as atsm,
                tc.tile_pool(name="ob", bufs=2) as obp,
            ):
                # one PSUM bank shared by the per-head rowsum
                # accumulators (f32, bytes 1024:1088) and the
                # transposed-reciprocal row (bf16, partition 0, bytes
                # 0:1024); all its groups run PE-serial.
                dentp = denps.tile([128, 512], f32, tag="den")
                dens = [dentp[:, 256:264], dentp[:, 264:272]]
                tp_t = dentp[0:1, 0:256].bitcast(bf16)

                def attn_chunk(p, c, mid_cb=None):
                    """Scores/exp (pipelined one tile ahead of PV), masks,
                    PV, rowsums, reciprocal.  mid_cb (sibling's finish) fires
                    after the first tile.  Returns the finish closure."""
                    b, h = p // 2, p % 2
                    jmax = 4 * (c + 1)
                    qtb, ktb = qt[h], kt[h]
                    o_ps = ops.tile([128, 512], f32, tag="o", name="o_ps")
                    den = dens[h]
                    pt_tiles = []

                    def trim(j):
                        return max(0, (j - 4 * c) * 128)

                    def emit_scores(m):
                        sp = sps.tile([128, 1024], f32, tag="s")
                        for i, j in ((0, 2 * m), (1, 2 * m + 1)):
                            t0 = trim(j)
                            nc.tensor.matmul(
                                sp[:, i * 512 + t0:(i + 1) * 512],
                                lhsT=ktb[:, b * T + j * 128: b * T + (j + 1) * 128],
                                rhs=qtb[:, b * T + c * 512 + t0:
                                        b * T + (c + 1) * 512],
                                start=True, stop=True)
                        pt_t = ptp.tile([128, 1024], bf16, tag="p")
                        t00 = trim(2 * m)
                        nc.scalar.activation(pt_t[:, t00:], sp[:, t00:],
                                             EXP, scale=SCALE)
                        pt_tiles.append(pt_t)

                    def emit_pv(m):
                        pt_t = pt_tiles[m]
                        for i, j in ((0, 2 * m), (1, 2 * m + 1)):
                            t0 = trim(j)
                            pth = pt_t[:, i * 512 + t0:(i + 1) * 512]
                            if j >= 4 * c:
                                nc.vector.tensor_mul(
                                    pt_t[:, i * 512 + t0:i * 512 + t0 + 128],
                                    pt_t[:, i * 512 + t0:i * 512 + t0 + 128],
                                    mask_sb[:, 0, 0:128])
                            nc.tensor.matmul(
                                o_ps[:, t0:],
                                lhsT=vres[:, b * 16 + j, h * 128:(h + 1) * 128],
                                rhs=pth,
                                start=(j == 0), stop=(j == jmax - 1))

                    emit_scores(0)
                    for m in range(jmax // 2):
                        if m + 1 < jmax // 2:
                            emit_scores(m + 1)
                        emit_pv(m)
                        if m == 0 and mid_cb is not None:
                            mid_cb()

                    for qq in range(4):
                        js = [j for j in range(jmax) if trim(j) <= qq * 128]
                        for jj, j in enumerate(js):
                            m, i = j // 2, j % 2
                            nc.tensor.matmul(
                                den[:, qq:qq + 1],
                                lhsT=pt_tiles[m][:, i * 512 + qq * 128:
                                                 i * 512 + qq * 128 + 128],
                                rhs=ones1,
                                start=(jj == 0), stop=(jj == len(js) - 1))
                    if c == 3:
                        for qq in range(4):
                            js = [j for j in range(12, 16)
                                  if trim(j) <= qq * 128]
                            for jj, j in enumerate(js):
                                m, i = j // 2, j % 2
                                nc.tensor.matmul(
                                    den[:, 4 + qq:5 + qq],
                                    lhsT=pt_tiles[m][:, i * 512 + qq * 128:
                                                     i * 512 + qq * 128 + 128],
                                    rhs=ones1,
                                    start=(jj == 0), stop=(jj == len(js) - 1))
                        for j in range(16):
                            m, i = j // 2, j % 2
                            nc.gpsimd.tensor_copy(
                                abar_raw[p][:, j:j + 1],
                                pt_tiles[m][:, i * 512 + 511: i * 512 + 512])

                    rec = atsm.tile([128, 4], bf16, tag="rec", name="rec")
                    with nc.allow_low_precision(
                            reason="softmax denominators applied in bf16"):
                        nc.vector.reciprocal(rec, den[:, 0:4])
                    if c == 3:
                        dexp = atsm.tile([128, 8], f32, tag="dexp",
                                         name="dexp")
                        nc.vector.tensor_copy(dexp, den[:, 0:8])
                        nc.sync.dma_start(out=dentd[p], in_=dexp)

                    def finish():
                        for qq in range(4):
                            nc.tensor.transpose(
                                tp_t[0:1, qq * 128:(qq + 1) * 128],
                                rec[:, qq:qq + 1], ident_sb)
                        rrow = atsm.tile([1, 512], bf16, tag="rrow",
                                         name="rrow")
                        nc.vector.tensor_copy(rrow, tp_t)
                        bc = atsm.tile([128, 512], bf16, tag="bc", name="bc")
                        nc.gpsimd.partition_broadcast(bc, rrow)
                        tn = atsm.tile([128, 512], bf16, tag="tn", name="tn")
                        nc.vector.scalar_tensor_tensor(
                            out=tn, in0=o_ps, scalar=S_OT, in1=bc,
                            op0=mybir.AluOpType.mult,
                            op1=mybir.AluOpType.mult)
                        nc.vector.tensor_copy(ot_hi[b][c][:, h, :], tn)
                        nc.vector.scalar_tensor_tensor(
                            out=ot_lo[b][c][:, h, :], in0=tn, scalar=1.0,
                            in1=ot_hi[b][c][:, h, :],
                            op0=mybir.AluOpType.mult,
                            op1=mybir.AluOpType.subtract)
                    return finish

                def oproj_chunk(b, c):
                    cg = slice((b * 4 + c) * 512, (b * 4 + c + 1) * 512)
                    oth, otl = ot_hi[b][c], ot_lo[b][c]
                    for quarter in range(4):
                        ob = obp.tile([128, 4, 512], bf16, tag="ob")
                        for fi in range(4):
                            fo = quarter * 4 + fi
                            fs = slice(fo * 128, (fo + 1) * 128)
                            pp = ops.tile([128, 512], f32, tag="o",
                                          name="pp")
                            for ti, (wsb, osb) in enumerate(
                                    ((wo_hi_sb, oth), (wo_hi_sb, otl),
                                     (wo_lo_sb, oth))):
                                nc.tensor.matmul(
                                    pp, lhsT=wsb[:, :, fs], rhs=osb,
                                    start=(ti == 0), stop=(ti == 2),
                                    perf_mode=DR)
                            if fi != 3:
                                nc.vector.tensor_scalar_mul(
                                    ob[:, fi, :], pp, DESC_O)
                            else:
                                nc.scalar.activation(
                                    ob[:, fi, :], pp, COPY, scale=DESC_O)
                        nc.sync.dma_start(
                            out=outT[quarter * 512:(quarter + 1) * 512, cg]
                            .rearrange("(nf p) t -> p nf t", p=128),
                            in_=ob)

                for b in range(B):
                    for c in range(4):
                        f0 = attn_chunk(b * 2 + 0, c)
                        f1 = attn_chunk(b * 2 + 1, c, mid_cb=f0)
                        f1()
                        if c > 0:
                            oproj_chunk(b, c - 1)
                    oproj_chunk(b, 3)
                    for hl in range(HL):
                        p = b * 2 + hl
                        nc.sync.dma_start(out=abard[p], in_=abar_raw[p])

    nc.compile()
    return nc


def _get_nc():
    if "nc" not in _NC_CACHE:
        _NC_CACHE["nc"] = build_nc()
    return _NC_CACHE["nc"]


def _hi_lo(x, dtype=F8):
    hi = np.asarray(x, np.float32).astype(dtype)
    lo = (np.asarray(x, np.float32) - hi.astype(np.float32)).astype(dtype)
    return hi, lo


def _host_inputs(hidden_states, q_w, k_w, v_w, o_w):
    """Per-core input dicts."""
    hsT = np.ascontiguousarray(hidden_states.reshape(BT, HID).T)
    hs_hi, hs_lo = _hi_lo(S_HS * hsT)
    inv = 10000.0 ** (-np.arange(64, dtype=np.float64) / 64.0)
    t = np.arange(T, dtype=np.float64)
    fr = t[None, :] * inv[:, None]  # [64, T]
    cosT = np.cos(np.concatenate([fr, fr], 0)).astype(np.float32)
    sinT = np.sin(np.concatenate([fr, fr], 0)).astype(np.float32)
    sinT[:64] *= -1.0  # sign-baked for swap-halves rotate
    cosT = cosT.astype(BF16)
    sinT = sinT.astype(BF16)
    masks = np.zeros((128, 4, 512), dtype=np.float32)
    kk = np.arange(128)[:, None]
    tt = np.arange(512)[None, :]
    for v in range(4):
        masks[:, v, :] = (tt >= 128 * v + kk).astype(np.float32)
    masks = masks.reshape(128, 2048).astype(BF16)
    ident = np.eye(128, dtype=np.float32).astype(BF16)

    in_maps = []
    for core in range(NCORES):
        rs = slice(core * 256, (core + 1) * 256)
        wo_hi, wo_lo = _hi_lo(S_WO * np.ascontiguousarray(o_w[:, rs].T))
        d = {
            "hs_hi": hs_hi,
            "hs_lo": hs_lo,
            "wo_hi": np.ascontiguousarray(
                wo_hi.reshape(2, 128, HID).transpose(1, 0, 2).reshape(128, -1)),
            "wo_lo": np.ascontiguousarray(
                wo_lo.reshape(2, 128, HID).transpose(1, 0, 2).reshape(128, -1)),
            "cosT": cosT,
            "sinT": sinT,
            "masks": masks,
            "ident": ident,
        }
        for nm, w in (("wq", q_w), ("wk", k_w), ("wv", v_w)):
            hi, lo = _hi_lo(S_W * np.ascontiguousarray(w[rs, :].T))
            # swizzle [(nf p), d] -> [p, nf*d] to make the DMA contiguous
            d[f"{nm}_hi"] = np.ascontiguousarray(
                hi.reshape(NF, 128, 256).transpose(1, 0, 2).reshape(128, -1))
            d[f"{nm}_lo"] = np.ascontiguousarray(
                lo.reshape(NF, 128, 256).transpose(1, 0, 2).reshape(128, -1))
        in_maps.append(d)
    return in_maps


def _epilogue(out, results, hidden_states, v_w, o_w):
    """Add the CaM rank-1 correction per (b, h) on host."""
    for core in range(NCORES):
        r = results[core]
        for p in range(4):
            b, hl = p // 2, p % 2
            h = core * HL + hl
            dent = np.asarray(r["dent"][p], np.float64)  # [128, 8]
            rowsum3 = dent[:, 0:4].T.reshape(512)  # t = 1536 + idx
            tails3 = dent[:, 4:8].T.reshape(512)
            a_exp = np.asarray(r["abar"][p], np.float64).T.reshape(2048)
            rs_last = max(float(rowsum3[511]), 1e-30)
            a_bar = a_exp / rs_last
            avg_w = max(float(np.mean(a_bar[WS:])), 1e-6)
            prob = float(np.clip(a_bar[EVICT] / avg_w, 0.0, 1.0))
            prob = float(np.nan_to_num(prob, nan=0.0, posinf=1.0, neginf=0.0))
            m = 1.0 if U_CONST[b, h] < prob else 0.0
            if m == 0.0:
                continue
            # exact v_e from fp32 inputs
            v_row = hidden_states[b, EVICT, :] @ v_w[h * D:(h + 1) * D, :].T
            v_e = v_row * (m / RB)  # [D]
            w_e = o_w[:, h * D:(h + 1) * D] @ v_e  # [HID]
            s_tail = np.zeros(T, dtype=np.float32)
            s_tail[WS:] = (tails3 / np.maximum(rowsum3, 1e-30)).astype(
                np.float32)
            out[b] += np.outer(s_tail, w_e).astype(np.float32)
    return out


def kernel(hidden_states, attention_mask, q_w, k_w, v_w, o_w):
    from concourse.bass_utils import run_bass_kernel_spmd

    nc = _get_nc()
    in_maps = _host_inputs(hidden_states, q_w, k_w, v_w, o_w)
    trace = bool(int(os.environ.get("BK_TRACE", "0")))
    res = run_bass_kernel_spmd(
        nc, in_maps, core_ids=list(range(NCORES)), trace=trace,
    )
    if trace and res.exec_time_ns is not None:
        print(f"HW exec time: {res.exec_time_ns} ns")
        _NC_CACHE["last_exec_ns"] = res.exec_time_ns
        _NC_CACHE["last_trace"] = res.instructions_and_trace
    results = res.results

    acc = np.zeros((HID, BT), dtype=np.float32)
    for core in range(NCORES):
        acc += np.asarray(results[core]["outT"], np.float32)
    out = np.ascontiguousarray(acc.T).reshape(B, T, HID)
    out = _epilogue(out, results, hidden_states, v_w, o_w)
    return out.astype(np.float32)


# revision 60
# speedup vs baseline: 1.0371x; 1.0371x over previous
"""Trainium2 Bass kernel for nn_LlamaAttention_cam (sparse_attention).

Sharding: 16 heads across 8 cores (2 heads/core), both batches per core.
Q/K/V projections column-parallel over heads, o_proj row-parallel (per-core
partial outputs summed on host). The CaM merge is a rank-1 correction
applied on host from tiny device-side statistics.

Key optimizations over the bf16 baseline (387us -> 285us cost-model time):
- QKV and o_proj matmuls run in error-compensated fp8: each operand is
  split host-side (or on the normalize path) into e4m3 hi + lo parts and
  the product takes 3 of the 4 cross terms as DoubleRow matmuls, giving
  0.75x the bf16 PE cost at ~bf16 accuracy (plain fp8 measured 5-10x
  over the error budget; hi+lo lands at ~1.3e-2 vs the 2e-2 gate).
- Softmax row-sums cost ~1 PE cycle per 128x128 P block: P quarters are
  the *stationary* operand with a 1-column ones vector moving (matmul
  cost scales with moving columns only).  Per-query reciprocals are
  PE-transposed into a partition-0 row and applied via one partition
  broadcast.
- Causal trimming at 128-column granularity on scores/exp/PV/rowsums.
- Scores/exp run one wide tile ahead of mask/PV; the sibling head's
  normalize chain and the previous chunk's o_proj are emitted inside the
  next attention chunk so the PE rarely waits on Act/DVE/Pool chains.
- DMAs are issued in deadline order (first hs chunk + hi weights first,
  cold constants last); weights are host-preswizzled so every transfer
  moves contiguous >=512B descriptors at full DMA rate.

Self-contained: hardcodes all shapes; takes full inputs, returns full output.
"""

import math
import os

import numpy as np
import ml_dtypes

B, T, HID, H = 2, 2048, 2048, 16
D = 128
NCORES = 8
HL = H // NCORES  # heads per core = 2
BT = B * T  # 4096
NF = HID // 128  # 16 f-tiles
SCALE = 1.0 / math.sqrt(D)
RB = int(0.25 * T)  # 512 recent budget
WS = T - RB  # 1536
EVICT = WS - 1  # 1535

S_HS = 8.0
S_W = 256.0
DESC = 1.0 / (S_HS * S_W)  # 1/2048
S_OT = 16.0
S_WO = 256.0
DESC_O = 1.0 / (S_OT * S_WO)  # 1/4096

# jax.random.uniform(jax.random.key(42), (2,16), float32); bernoulli(key,p) == u < p
U_CONST = np.array(
    [[0.59400654, 0.43801308, 0.6285691, 0.00791204, 0.27834702,
      0.7976179, 0.8521497, 0.9625306, 0.67656493, 0.11104441,
      0.4959929, 0.7311437, 0.18970704, 0.1544199, 0.03802836,
      0.33559263],
     [0.92825687, 0.6123972, 0.49262476, 0.733806, 0.18920851,
      0.15386605, 0.037136197, 0.32930005, 0.9372028, 0.5957513,
      0.4615929, 0.6695677, 0.07019377, 0.39408123, 0.55786455,
      0.35412872]], dtype=np.float32)

BF16 = ml_dtypes.bfloat16
F8 = ml_dtypes.float8_e4m3

_NC_CACHE = {}


def build_nc():
    import concourse.bacc as bacc
    import concourse.mybir as mybir
    import concourse.tile as tile

    f32 = mybir.dt.float32
    bf16 = mybir.dt.bfloat16
    fp8 = mybir.dt.float8e4
    EXP = mybir.ActivationFunctionType.Exp
    COPY = mybir.ActivationFunctionType.Copy
    DR = mybir.MatmulPerfMode.DoubleRow

    nc = bacc.Bacc("TRN2", target_bir_lowering=False, debug=False)

    hs_hi = nc.dram_tensor("hs_hi", [HID, BT], fp8, kind="ExternalInput")
    hs_lo = nc.dram_tensor("hs_lo", [HID, BT], fp8, kind="ExternalInput")
    w8 = {}
    for nm in ("wq", "wk", "wv"):
        for part in ("hi", "lo"):
            # host pre-swizzled to the SBUF layout: [p, nf*256] contiguous
            w8[f"{nm}_{part}"] = nc.dram_tensor(
                f"{nm}_{part}", [128, NF * 256], fp8, kind="ExternalInput")
    wo_hi = nc.dram_tensor("wo_hi", [128, 2 * HID], fp8, kind="ExternalInput")
    wo_lo = nc.dram_tensor("wo_lo", [128, 2 * HID], fp8, kind="ExternalInput")
    cosd = nc.dram_tensor("cosT", [128, T], bf16, kind="ExternalInput")
    sind = nc.dram_tensor("sinT", [128, T], bf16, kind="ExternalInput")
    maskd = nc.dram_tensor("masks", [128, 2048], bf16, kind="ExternalInput")
    identd = nc.dram_tensor("ident", [128, 128], bf16, kind="ExternalInput")

    outT = nc.dram_tensor("outT", [HID, BT], bf16, kind="ExternalOutput")
    abard = nc.dram_tensor("abar", [4, 128, 16], bf16, kind="ExternalOutput")
    # per (p): cols 0:4 = chunk-3 rowsums, cols 4:8 = chunk-3 tail sums
    dentd = nc.dram_tensor("dent", [4, 128, 8], f32, kind="ExternalOutput")

    with tile.TileContext(nc) as tc:
        with (
            tc.tile_pool(name="singles", bufs=1) as singles,
            tc.tile_pool(name="res", bufs=1) as res,
            tc.tile_pool(name="stats", bufs=1) as stats,
        ):
            # --- allocate constant tiles (DMAs emitted below, ordered) ---
            w8_sb = {key: singles.tile([128, NF, 256], fp8, tag=key, name=key)
                     for key in w8}
            wo_hi_sb = singles.tile([128, 2, HID], fp8, tag="wohi")
            wo_lo_sb = singles.tile([128, 2, HID], fp8, tag="wolo")
            cos_sb = singles.tile([128, T], bf16, tag="cos")
            sin_sb = singles.tile([128, T], bf16, tag="sin")
            mask_sb = singles.tile([128, 4, 512], bf16, tag="mask")
            ident_sb = singles.tile([128, 128], bf16, tag="ident")
            ones1 = singles.tile([128, 1], bf16, tag="ones1")
            nc.vector.memset(ones1, 1.0)

            # --- residents ---
            qt = [res.tile([128, BT], bf16, tag=f"qt{h}", name=f"qt{h}")
                  for h in range(HL)]
            kt = [res.tile([128, BT], bf16, tag=f"kt{h}", name=f"kt{h}")
                  for h in range(HL)]
            vres = res.tile([128, 32, 256], bf16, tag="vres")
            ot_hi = [[res.tile([128, 2, 512], fp8, tag=f"oth{b}_{c}",
                               name=f"oth{b}_{c}")
                      for c in range(4)] for b in range(2)]
            ot_lo = [[res.tile([128, 2, 512], fp8, tag=f"otl{b}_{c}",
                               name=f"otl{b}_{c}")
                      for c in range(4)] for b in range(2)]
            abar_raw = [stats.tile([128, 16], bf16, tag=f"ab{p}",
                                   name=f"ab{p}") for p in range(4)]

            # ============ Phase 1: QKV projections (comp-fp8) + RoPE ========
            with (
                tc.tile_pool(name="hsp", bufs=2) as hsp,
                tc.tile_pool(name="rope", bufs=4) as rope,
                tc.tile_pool(name="qkps", bufs=5, space="PSUM") as qkps,
                tc.tile_pool(name="vps", bufs=3, space="PSUM") as vps,
            ):
                def load_hs_half(c, which):
                    cs = slice(c * 512, (c + 1) * 512)
                    src = hs_hi if which == "hshi" else hs_lo
                    t = hsp.tile([128, NF, 512], fp8, tag=which, name=which)
                    nc.sync.dma_start(
                        out=t,
                        in_=src[:, cs].rearrange("(nf p) t -> p nf t", p=128))
                    return t

                def load_w(key):
                    nc.sync.dma_start(
                        out=w8_sb[key],
                        in_=w8[key].rearrange("p (nf d) -> p nf d", d=256))

                # DMA order tuned so the PE never starves at kernel start.
                # All transfers share the DMA engines, so everything goes on
                # the sync queue in deadline order: hs_hi(c0), hi weights,
                # lo weights, hs_lo(c0), hs(c1), then the cold constants
                # (cos/sin feed only the DVE rope; masks/ident/wo are needed
                # far later).
                def load_trig(v):
                    vs = slice(v * 512, (v + 1) * 512)
                    nc.sync.dma_start(out=cos_sb[:, vs], in_=cosd[:, vs])
                    nc.sync.dma_start(out=sin_sb[:, vs], in_=sind[:, vs])

                nc.sync.dma_start(
                    out=w8_sb["wq_hi"][:, 0:4, :],
                    in_=w8["wq_hi"][:, 0:1024].rearrange(
                        "p (nf d) -> p nf d", d=256))
                cs0 = slice(0, 512)
                hi0 = hsp.tile([128, NF, 512], fp8, tag="hshi", name="hshi")
                nc.sync.dma_start(
                    out=hi0[:, 0:4, :],
                    in_=hs_hi[0:512, cs0].rearrange(
                        "(nf p) t -> p nf t", p=128))
                nc.sync.dma_start(
                    out=w8_sb["wq_hi"][:, 4:8, :],
                    in_=w8["wq_hi"][:, 1024:2048].rearrange(
                        "p (nf d) -> p nf d", d=256))
                nc.sync.dma_start(
                    out=hi0[:, 4:8, :],
                    in_=hs_hi[512:1024, cs0].rearrange(
                        "(nf p) t -> p nf t", p=128))
                nc.sync.dma_start(
                    out=hi0[:, 8:16, :],
                    in_=hs_hi[1024:2048, cs0].rearrange(
                        "(nf p) t -> p nf t", p=128))
                nc.sync.dma_start(
                    out=w8_sb["wq_hi"][:, 8:16, :],
                    in_=w8["wq_hi"][:, 2048:4096].rearrange(
                        "p (nf d) -> p nf d", d=256))
                for key in ("wk_hi", "wv_hi", "wq_lo", "wk_lo", "wv_lo"):
                    load_w(key)
                load_trig(0)
                lo0 = load_hs_half(0, "hslo")
                hs_tiles = (hi0, lo0)
                load_trig(1)
                hs_next = (load_hs_half(1, "hshi"), load_hs_half(1, "hslo"))
                load_trig(2)
                load_trig(3)
                nc.sync.dma_start(out=ident_sb, in_=identd[:, :])
                nc.sync.dma_start(
                    out=mask_sb, in_=maskd.rearrange("p (v t) -> p v t", v=4))
                nc.sync.dma_start(
                    out=wo_hi_sb, in_=wo_hi.rearrange("p (kt f) -> p kt f",
                                                      f=HID))
                nc.sync.dma_start(
                    out=wo_lo_sb, in_=wo_lo.rearrange("p (kt f) -> p kt f",
                                                      f=HID))

                for c in range(8):
                    cs = slice(c * 512, (c + 1) * 512)
                    hs_hi_t, hs_lo_t = hs_tiles
                    hs_tiles = hs_next
                    if c < 6:
                        hs_next = (load_hs_half(c + 2, "hshi"),
                                   load_hs_half(c + 2, "hslo"))
                    # Three passes with open PSUM groups: A1 = w_hi*hs_hi,
                    # A2 = w_lo*hs_hi, B = w_hi*hs_lo.  This relaxes the
                    # arrival deadlines of the lo weights and hs_lo DMAs.
                    qk_ps = {}
                    v_ps = []
                    for h in range(HL):
                        hsl = slice(h * 128, (h + 1) * 128)
                        for nm in ("wq", "wk"):
                            ps = qkps.tile([128, 512], f32, tag="qk")
                            qk_ps[(nm, h)] = ps
                            for s in range(8):
                                nc.tensor.matmul(
                                    ps,
                                    lhsT=w8_sb[f"{nm}_hi"][:, 2 * s:2 * s + 2, hsl],
                                    rhs=hs_hi_t[:, 2 * s:2 * s + 2, :],
                                    start=(s == 0), stop=False, perf_mode=DR)
                    for sblk in range(4):
                        tsl = slice(sblk * 128, (sblk + 1) * 128)
                        vp = vps.tile([128, 512], f32, tag="v")
                        v_ps.append(vp)
                        for s in range(8):
                            nc.tensor.matmul(
                                vp[:, 0:256],
                                lhsT=hs_hi_t[:, 2 * s:2 * s + 2, tsl],
                                rhs=w8_sb["wv_hi"][:, 2 * s:2 * s + 2, :],
                                start=(s == 0), stop=False, perf_mode=DR)
                    # pass A2
                    for h in range(HL):
                        hsl = slice(h * 128, (h + 1) * 128)
                        for nm in ("wq", "wk"):
                            for s in range(8):
                                nc.tensor.matmul(
                                    qk_ps[(nm, h)],
                                    lhsT=w8_sb[f"{nm}_lo"][:, 2 * s:2 * s + 2, hsl],
                                    rhs=hs_hi_t[:, 2 * s:2 * s + 2, :],
                                    start=False, stop=False, perf_mode=DR)
                    for sblk in range(4):
                        tsl = slice(sblk * 128, (sblk + 1) * 128)
                        for s in range(8):
                            nc.tensor.matmul(
                                v_ps[sblk][:, 0:256],
                                lhsT=hs_hi_t[:, 2 * s:2 * s + 2, tsl],
                                rhs=w8_sb["wv_lo"][:, 2 * s:2 * s + 2, :],
                                start=False, stop=False, perf_mode=DR)
                    # pass B + RoPE / V-copy epilogues
                    tl = slice((c % 4) * 512, (c % 4) * 512 + 512)
                    for h in range(HL):
                        hsl = slice(h * 128, (h + 1) * 128)
                        for nm, dest in (("wq", qt[h]), ("wk", kt[h])):
                            ps = qk_ps[(nm, h)]
                            for s in range(8):
                                nc.tensor.matmul(
                                    ps,
                                    lhsT=w8_sb[f"{nm}_hi"][:, 2 * s:2 * s + 2, hsl],
                                    rhs=hs_lo_t[:, 2 * s:2 * s + 2, :],
                                    start=False, stop=(s == 7), perf_mode=DR)
                            qf = rope.tile([128, 512], bf16, tag="qf")
                            if c == 7:
                                nc.vector.tensor_scalar_mul(qf, ps, DESC)
                            else:
                                nc.scalar.activation(qf, ps, COPY, scale=DESC)
                            rot = rope.tile([128, 512], bf16, tag="rot")
                            nc.gpsimd.dma_start(out=rot[0:64, :], in_=qf[64:128, :])
                            nc.gpsimd.dma_start(out=rot[64:128, :], in_=qf[0:64, :])
                            t1 = rope.tile([128, 512], bf16, tag="t1")
                            nc.vector.tensor_mul(t1, rot, sin_sb[:, tl])
                            t2 = rope.tile([128, 512], bf16, tag="t2")
                            nc.vector.tensor_mul(t2, qf, cos_sb[:, tl])
                            nc.vector.tensor_add(dest[:, cs], t1, t2)
                    for sblk in range(4):
                        tsl = slice(sblk * 128, (sblk + 1) * 128)
                        vp = v_ps[sblk]
                        for s in range(8):
                            nc.tensor.matmul(
                                vp[:, 0:256],
                                lhsT=hs_lo_t[:, 2 * s:2 * s + 2, tsl],
                                rhs=w8_sb["wv_hi"][:, 2 * s:2 * s + 2, :],
                                start=False, stop=(s == 7), perf_mode=DR)
                        if c == 7:
                            nc.vector.tensor_scalar_mul(
                                vres[:, c * 4 + sblk, :], vp[:, 0:256], DESC)
                        else:
                            nc.scalar.activation(
                                vres[:, c * 4 + sblk, :], vp[:, 0:256], COPY,
                                scale=DESC)

            # ========== Phase 2+3: attention + interleaved o_proj ==========
            with (
                tc.tile_pool(name="sps", bufs=2, space="PSUM") as sps,
                tc.tile_pool(name="ops", bufs=3, space="PSUM") as ops,
                tc.tile_pool(name="denps", bufs=1, space="PSUM") as denps,
                tc.tile_pool(name="pt", bufs=16) as ptp,
                tc.tile_pool(name="att_sm", bufs=5) as atsm,
                tc.tile_pool(name="ob", bufs=4) as obp,
            ):
                # one PSUM bank shared by the per-head rowsum
                # accumulators (f32, bytes 1024:1088) and the
                # transposed-reciprocal row (bf16, partition 0, bytes
                # 0:1024); all its groups run PE-serial.
                dentp = denps.tile([128, 512], f32, tag="den")
                dens = [dentp[:, 256:264], dentp[:, 264:272]]
                tp_t = dentp[0:1, 0:256].bitcast(bf16)

                def attn_chunk(p, c, mids=()):
                    """Scores/exp (pipelined one tile ahead of PV), masks,
                    PV.  mids (the sibling head's deferred tail/finish work)
                    fire one per PV step so the PE never head-of-line blocks
                    on DVE mask-muls.  Returns (tail, finish): tail emits
                    rowsums + reciprocal (+ c3 exports); finish emits the
                    PE transpose + broadcast + normalize into ot8."""
                    b, h = p // 2, p % 2
                    jmax = 4 * (c + 1)
                    qtb, ktb = qt[h], kt[h]
                    o_ps = ops.tile([128, 512], f32, tag="o", name="o_ps")
                    den = dens[h]
                    pt_tiles = []
                    cell = {}

                    def trim(j):
                        return max(0, (j - 4 * c) * 128)

                    def emit_scores(m):
                        sp = sps.tile([128, 1024], f32, tag="s")
                        for i, j in ((0, 2 * m), (1, 2 * m + 1)):
                            t0 = trim(j)
                            nc.tensor.matmul(
                                sp[:, i * 512 + t0:(i + 1) * 512],
                                lhsT=ktb[:, b * T + j * 128: b * T + (j + 1) * 128],
                                rhs=qtb[:, b * T + c * 512 + t0:
                                        b * T + (c + 1) * 512],
                                start=True, stop=True)
                        pt_t = ptp.tile([128, 1024], bf16, tag="p")
                        t00 = trim(2 * m)
                        nc.scalar.activation(pt_t[:, t00:], sp[:, t00:],
                                             EXP, scale=SCALE)
                        pt_tiles.append(pt_t)

                    def emit_pv(m):
                        pt_t = pt_tiles[m]
                        for i, j in ((0, 2 * m), (1, 2 * m + 1)):
                            t0 = trim(j)
                            pth = pt_t[:, i * 512 + t0:(i + 1) * 512]
                            if j >= 4 * c:
                                nc.vector.tensor_mul(
                                    pt_t[:, i * 512 + t0:i * 512 + t0 + 128],
                                    pt_t[:, i * 512 + t0:i * 512 + t0 + 128],
                                    mask_sb[:, 0, 0:128])
                            nc.tensor.matmul(
                                o_ps[:, t0:],
                                lhsT=vres[:, b * 16 + j, h * 128:(h + 1) * 128],
                                rhs=pth,
                                start=(j == 0), stop=(j == jmax - 1))

                    ex = list(mids)
                    emit_scores(0)
                    for m in range(jmax // 2):
                        if m + 1 < jmax // 2:
                            emit_scores(m + 1)
                        emit_pv(m)
                        if ex:
                            ex.pop(0)()
                    while ex:
                        ex.pop(0)()

                    def tail():
                        for qq in range(4):
                            js = [j for j in range(jmax)
                                  if trim(j) <= qq * 128]
                            for jj, j in enumerate(js):
                                m, i = j // 2, j % 2
                                nc.tensor.matmul(
                                    den[:, qq:qq + 1],
                                    lhsT=pt_tiles[m][:, i * 512 + qq * 128:
                                                     i * 512 + qq * 128 + 128],
                                    rhs=ones1,
                                    start=(jj == 0),
                                    stop=(jj == len(js) - 1))
                        if c == 3:
                            for qq in range(4):
                                js = [j for j in range(12, 16)
                                      if trim(j) <= qq * 128]
                                for jj, j in enumerate(js):
                                    m, i = j // 2, j % 2
                                    nc.tensor.matmul(
                                        den[:, 4 + qq:5 + qq],
                                        lhsT=pt_tiles[m][:, i * 512 + qq * 128:
                                                         i * 512 + qq * 128 + 128],
                                        rhs=ones1,
                                        start=(jj == 0),
                                        stop=(jj == len(js) - 1))
                            for j in range(16):
                                m, i = j // 2, j % 2
                                nc.gpsimd.tensor_copy(
                                    abar_raw[p][:, j:j + 1],
                                    pt_tiles[m][:, i * 512 + 511:
                                                i * 512 + 512])
                        rec = atsm.tile([128, 4], bf16, tag="rec",
                                        name="rec")
                        with nc.allow_low_precision(
                                reason="softmax denominators in bf16"):
                            nc.vector.reciprocal(rec, den[:, 0:4])
                        cell["rec"] = rec
                        if c == 3:
                            dexp = atsm.tile([128, 8], f32, tag="dexp",
                                             name="dexp")
                            nc.vector.tensor_copy(dexp, den[:, 0:8])
                            nc.sync.dma_start(out=dentd[p], in_=dexp)

                    def finish():
                        rec = cell["rec"]
                        for qq in range(4):
                            nc.tensor.transpose(
                                tp_t[0:1, qq * 128:(qq + 1) * 128],
                                rec[:, qq:qq + 1], ident_sb)
                        rrow = atsm.tile([1, 512], bf16, tag="rrow",
                                         name="rrow")
                        nc.vector.tensor_copy(rrow, tp_t)
                        bc = atsm.tile([128, 512], bf16, tag="bc", name="bc")
                        nc.gpsimd.partition_broadcast(bc, rrow)
                        tn = atsm.tile([128, 512], bf16, tag="tn", name="tn")
                        nc.vector.scalar_tensor_tensor(
                            out=tn, in0=o_ps, scalar=S_OT, in1=bc,
                            op0=mybir.AluOpType.mult,
                            op1=mybir.AluOpType.mult)
                        nc.vector.tensor_copy(ot_hi[b][c][:, h, :], tn)
                        nc.vector.scalar_tensor_tensor(
                            out=ot_lo[b][c][:, h, :], in0=tn, scalar=1.0,
                            in1=ot_hi[b][c][:, h, :],
                            op0=mybir.AluOpType.mult,
                            op1=mybir.AluOpType.subtract)
                    return tail, finish

                def oproj_chunk(b, c):
                    cg = slice((b * 4 + c) * 512, (b * 4 + c + 1) * 512)
                    oth, otl = ot_hi[b][c], ot_lo[b][c]
                    for quarter in range(4):
                        ob = obp.tile([128, 4, 512], bf16, tag="ob")
                        for fi in range(4):
                            fo = quarter * 4 + fi
                            fs = slice(fo * 128, (fo + 1) * 128)
                            pp = ops.tile([128, 512], f32, tag="o",
                                          name="pp")
                            for ti, (wsb, osb) in enumerate(
                                    ((wo_hi_sb, oth), (wo_hi_sb, otl),
                                     (wo_lo_sb, oth))):
                                nc.tensor.matmul(
                                    pp, lhsT=wsb[:, :, fs], rhs=osb,
                                    start=(ti == 0), stop=(ti == 2),
                                    perf_mode=DR)
                            if fi != 3:
                                nc.vector.tensor_scalar_mul(
                                    ob[:, fi, :], pp, DESC_O)
                            else:
                                nc.scalar.activation(
                                    ob[:, fi, :], pp, COPY, scale=DESC_O)
                        nc.sync.dma_start(
                            out=outT[quarter * 512:(quarter + 1) * 512, cg]
                            .rearrange("(nf p) t -> p nf t", p=128),
                            in_=ob)

                for b in range(B):
                    for c in range(4):
                        t0, f0 = attn_chunk(b * 2 + 0, c)
                        t1, f1 = attn_chunk(b * 2 + 1, c, mids=[t0, f0])
                        t1()
                        f1()
                        if c > 0:
                            oproj_chunk(b, c - 1)
                    oproj_chunk(b, 3)
                    for hl in range(HL):
                        p = b * 2 + hl
                        nc.sync.dma_start(out=abard[p], in_=abar_raw[p])

    nc.compile()
    return nc


def _get_nc():
    if "nc" not in _NC_CACHE:
        _NC_CACHE["nc"] = build_nc()
    return _NC_CACHE["nc"]


def _hi_lo(x, dtype=F8):
    hi = np.asarray(x, np.float32).astype(dtype)
    lo = (np.asarray(x, np.float32) - hi.astype(np.float32)).astype(dtype)
    return hi, lo


def _host_inputs(hidden_states, q_w, k_w, v_w, o_w):
    """Per-core input dicts."""
    hsT = np.ascontiguousarray(hidden_states.reshape(BT, HID).T)
    hs_hi, hs_lo = _hi_lo(S_HS * hsT)
    inv = 10000.0 ** (-np.arange(64, dtype=np.float64) / 64.0)
    t = np.arange(T, dtype=np.float64)
    fr = t[None, :] * inv[:, None]  # [64, T]
    cosT = np.cos(np.concatenate([fr, fr], 0)).astype(np.float32)
    sinT = np.sin(np.concatenate([fr, fr], 0)).astype(np.float32)
    sinT[:64] *= -1.0  # sign-baked for swap-halves rotate
    cosT = cosT.astype(BF16)
    sinT = sinT.astype(BF16)
    masks = np.zeros((128, 4, 512), dtype=np.float32)
    kk = np.arange(128)[:, None]
    tt = np.arange(512)[None, :]
    for v in range(4):
        masks[:, v, :] = (tt >= 128 * v + kk).astype(np.float32)
    masks = masks.reshape(128, 2048).astype(BF16)
    ident = np.eye(128, dtype=np.float32).astype(BF16)

    in_maps = []
    for core in range(NCORES):
        rs = slice(core * 256, (core + 1) * 256)
        wo_hi, wo_lo = _hi_lo(S_WO * np.ascontiguousarray(o_w[:, rs].T))
        d = {
            "hs_hi": hs_hi,
            "hs_lo": hs_lo,
            "wo_hi": np.ascontiguousarray(
                wo_hi.reshape(2, 128, HID).transpose(1, 0, 2).reshape(128, -1)),
            "wo_lo": np.ascontiguousarray(
                wo_lo.reshape(2, 128, HID).transpose(1, 0, 2).reshape(128, -1)),
            "cosT": cosT,
            "sinT": sinT,
            "masks": masks,
            "ident": ident,
        }
        for nm, w in (("wq", q_w), ("wk", k_w), ("wv", v_w)):
            hi, lo = _hi_lo(S_W * np.ascontiguousarray(w[rs, :].T))
            # swizzle [(nf p), d] -> [p, nf*d] to make the DMA contiguous
            d[f"{nm}_hi"] = np.ascontiguousarray(
                hi.reshape(NF, 128, 256).transpose(1, 0, 2).reshape(128, -1))
            d[f"{nm}_lo"] = np.ascontiguousarray(
                lo.reshape(NF, 128, 256).transpose(1, 0, 2).reshape(128, -1))
        in_maps.append(d)
    return in_maps


def _epilogue(out, results, hidden_states, v_w, o_w):
    """Add the CaM rank-1 correction per (b, h) on host."""
    for core in range(NCORES):
        r = results[core]
        for p in range(4):
            b, hl = p // 2, p % 2
            h = core * HL + hl
            dent = np.asarray(r["dent"][p], np.float64)  # [128, 8]
            rowsum3 = dent[:, 0:4].T.reshape(512)  # t = 1536 + idx
            tails3 = dent[:, 4:8].T.reshape(512)
            a_exp = np.asarray(r["abar"][p], np.float64).T.reshape(2048)
            rs_last = max(float(rowsum3[511]), 1e-30)
            a_bar = a_exp / rs_last
            avg_w = max(float(np.mean(a_bar[WS:])), 1e-6)
            prob = float(np.clip(a_bar[EVICT] / avg_w, 0.0, 1.0))
            prob = float(np.nan_to_num(prob, nan=0.0, posinf=1.0, neginf=0.0))
            m = 1.0 if U_CONST[b, h] < prob else 0.0
            if m == 0.0:
                continue
            # exact v_e from fp32 inputs
            v_row = hidden_states[b, EVICT, :] @ v_w[h * D:(h + 1) * D, :].T
            v_e = v_row * (m / RB)  # [D]
            w_e = o_w[:, h * D:(h + 1) * D] @ v_e  # [HID]
            s_tail = np.zeros(T, dtype=np.float32)
            s_tail[WS:] = (tails3 / np.maximum(rowsum3, 1e-30)).astype(
                np.float32)
            out[b] += np.outer(s_tail, w_e).astype(np.float32)
    return out


def kernel(hidden_states, attention_mask, q_w, k_w, v_w, o_w):
    from concourse.bass_utils import run_bass_kernel_spmd

    nc = _get_nc()
    in_maps = _host_inputs(hidden_states, q_w, k_w, v_w, o_w)
    trace = bool(int(os.environ.get("BK_TRACE", "0")))
    res = run_bass_kernel_spmd(
        nc, in_maps, core_ids=list(range(NCORES)), trace=trace,
    )
    if trace and res.exec_time_ns is not None:
        print(f"HW exec time: {res.exec_time_ns} ns")
        _NC_CACHE["last_exec_ns"] = res.exec_time_ns
        _NC_CACHE["last_trace"] = res.instructions_and_trace
    results = res.results

    acc = np.zeros((HID, BT), dtype=np.float32)
    for core in range(NCORES):
        acc += np.asarray(results[core]["outT"], np.float32)
    out = np.ascontiguousarray(acc.T).reshape(B, T, HID)
    out = _epilogue(out, results, hidden_states, v_w, o_w)
    return out.astype(np.float32)
